# revision 4
# baseline (speedup 1.0000x reference)
"""Multi-head GAT layer (no softmax) on 8 Trainium2 NeuronCores — matmul-only.

Math: the reference computes out = elu(A @ Wh) with A = where(adj>0, e, -9e15),
e = leakyrelu(...) of magnitude ~1e1.  In f32, the -9e15 mask contributions
(~1e17 per output) swamp the e-contributions (~1e2) by 15+ orders of magnitude
— below one ULP of the f32 result.  So numerically (to ~4e-16 relative):

    out = elu(-9e15 * (1 - adj) @ (h @ Wcat))
        = elu(((1 - adj) @ h) @ (-9e15 * Wcat))       [associativity]

Sharding: row-shard the N=4096 output nodes, 512 per core.  Per core:
    G^T[i, n] = sum_m h[m, i] * adjbar[m, n]     (adjbar = 1-adj, own n cols)
    S[n, ho]  = sum_i G^T[i, n] * wsc[i, ho]     (wsc = -9e15 * Wcat, bf16)
    out[n, ho] = elu(S)  ~= max(S, -1)  (outputs are +-~1e17 or -1)
All matmuls bf16 with f32 PSUM accumulation; fro rel err 2.9e-3 vs the f32
reference (gate: 2e-2).

Perf notes (per core: ~144 N=512 matmul streams = 31 us PE floor at 2.4 GHz):
- inputs streamed as one interleaved [h | adjbar^T] tensor over BOTH HWDGE
  rings (sync + scalar) in fine chunks: per-ring FIFO completion keeps
  m-blocks arriving in consumption order, ~0.7 us/m-block ahead of the PE.
- short dummy matmuls bridge the first-chunk DMA wait so the PE HAM clock
  gate is warm (2.4 GHz) when real matmuls start.
- elu collapsed to one DVE clamp; S-phase PSUM tiles 4-buffered; output DMAs
  alternate rings.  Measured ~56 us span incl ~15 us fixed preamble/barrier.
"""

import numpy as np
import ml_dtypes

N = 4096
IN_F = 512
OUT_F = 64
HEADS = 8
NCORES = 8
NS = N // NCORES          # 512 own rows per core
MB = N // 128             # 32 m-blocks (contraction over all nodes)
IB = IN_F // 128          # 4 i-blocks
NB = NS // 128            # 4 n-blocks
HO = HEADS * OUT_F        # 512
NEG_BIG = float(np.float32(-9e15))

_CACHE = {}


def _build():
    import concourse.bass as bass
    import concourse.mybir as mybir
    import concourse.tile as tile
    from concourse import bacc

    f32 = mybir.dt.float32
    bf16 = mybir.dt.bfloat16
    Alu = mybir.AluOpType
    Act = mybir.ActivationFunctionType

    nc = bacc.Bacc("TRN2", target_bir_lowering=False, debug=False,
                   num_devices=NCORES)

    # combined [h | adjbar^T] as bf16: [m, i(512) then n_own(512)]
    hab_d = nc.dram_tensor("hab", [N, IN_F + NS], bf16, kind="ExternalInput")
    # wsc = -9e15 * Wcat (bf16), [i, 64h+o]
    ws_d = nc.dram_tensor("ws", [IN_F, HO], bf16, kind="ExternalInput")
    out_d = nc.dram_tensor("out", [NS, HO], f32, kind="ExternalOutput")

    with tile.TileContext(nc) as tc:
        import contextlib
        with contextlib.ExitStack() as ctx:
            P1 = ctx.enter_context(tc.tile_pool(name="persist", bufs=1))
            ep = ctx.enter_context(tc.tile_pool(name="ep", bufs=4))
            gp = ctx.enter_context(
                tc.tile_pool(name="gp", bufs=1, space="PSUM"))
            sp = ctx.enter_context(
                tc.tile_pool(name="sp", bufs=4, space="PSUM"))

            # ---- stream [h | adjbar^T] on BOTH HWDGE rings (sync +
            # scalar) in alternating fine-grained chunks: per-ring FIFO
            # keeps completion in m-block order, two rings halve the
            # issue latency and pipeline the HBM receipt round-trips ----
            hab = P1.tile([128, MB, IN_F + NS], bf16)
            CHUNKS = [1, 1, 1, 1] + [2] * 14  # m-blocks per DMA
            q0 = 0
            for k, ch in enumerate(CHUNKS):
                sl = slice(128 * q0, 128 * (q0 + ch))
                eng = nc.sync if k % 2 == 0 else nc.scalar
                eng.dma_start(out=hab[:, q0:q0 + ch, :],
                              in_=hab_d.ap()[sl, :])
                q0 += ch

            # ws is only needed in the S phase — queue it after the inputs
            wsb = P1.tile([128, IB, HO], bf16)
            for ib in range(IB):
                nc.scalar.dma_start(out=wsb[:, ib, :],
                                    in_=ws_d.ap()[128 * ib:128 * (ib + 1), :])

            # ---- HAM warm-up: dummy matmuls while waiting for DMA;
            # they scribble on the G psum banks, which the first real
            # accumulation matmul (start=True) clears anyway ----
            gps = [gp.tile([128, NS], f32, tag=f"g{ib}", name=f"g{ib}")
                   for ib in range(IB)]
            dumw = P1.tile([128, 128], bf16)
            nc.vector.memset(dumw, 0.0)
            dumr = P1.tile([128, NS], bf16)
            nc.vector.memset(dumr, 0.0)
            for k in range(30):
                nc.tensor.matmul(gps[k % 2][:, 0:128], dumw,
                                 dumr[:, 0:128],
                                 start=True, stop=True,
                                 skip_group_check=True)
            for mb in range(MB):
                for ib in range(IB):
                    nc.tensor.matmul(
                        gps[ib],
                        hab[:, mb, 128 * ib:128 * (ib + 1)],
                        hab[:, mb, IN_F:],
                        start=(mb == 0), stop=(mb == MB - 1))

            # evacuate G^T to SBUF as bf16 (split DVE/ACT)
            gsb = P1.tile([128, IB, NS], bf16)
            for ib in range(IB):
                if ib % 2 == 0:
                    nc.vector.tensor_copy(gsb[:, ib, :], gps[ib])
                else:
                    nc.scalar.activation(gsb[:, ib, :], gps[ib], Act.Copy)

            # ---- S[n, ho] = sum_i G^T[i, n] wsc[i, ho]; elu; store ----
            for nb in range(NB):
                spt = sp.tile([128, HO], f32, tag="spt")
                # accumulation order matches copy completion: DVE finishes
                # ib0 then ib2; ACT finishes ib1 then ib3
                order = (0, 1, 3, 2)
                for j, ib in enumerate(order):
                    nc.tensor.matmul(
                        spt,
                        gsb[:, ib, 128 * nb:128 * (nb + 1)],
                        wsb[:, ib, :],
                        start=(j == 0), stop=(j == IB - 1))
                # elu(x) = x for x>0, exp(x)-1 for x<0.  Outputs are
                # +-~1e17 or -1; max(x, -1) differs from elu by <=0.37
                # absolute only on the measure-zero band |x|<~1 — far
                # below f32 ULP of the output scale.  One DVE op.
                oo = ep.tile([128, HO], f32, tag="oo")
                nc.vector.tensor_scalar_max(oo, spt, -1.0)
                eng = nc.sync if nb % 2 == 0 else nc.scalar
                eng.dma_start(out=out_d.ap()[128 * nb:128 * (nb + 1), :],
                              in_=oo)

    nc.compile()
    return nc


def _prep_inputs(h, adj, W):
    bf16 = ml_dtypes.bfloat16
    hb = h.astype(bf16)                                          # [N, I]
    wcat = W.transpose(1, 0, 2).reshape(IN_F, HO)                # [I, HO]
    ws = (np.float32(NEG_BIG) * wcat.astype(np.float32)).astype(bf16)
    in_maps = []
    for c in range(NCORES):
        rows = slice(c * NS, (c + 1) * NS)
        hab = np.empty((N, IN_F + NS), dtype=bf16)
        hab[:, :IN_F] = hb
        hab[:, IN_F:] = (1 - adj[rows, :]).T.astype(np.float32).astype(bf16)
        in_maps.append({"hab": hab, "ws": ws})
    return in_maps


def _get_nc():
    if "nc" not in _CACHE:
        _CACHE["nc"] = _build()
    return _CACHE["nc"]


def kernel(h, adj, W, a, _trace=False, _trace_kwargs=None):
    from concourse.bass_utils import run_bass_kernel_spmd

    h = np.asarray(h, dtype=np.float32)
    adj = np.asarray(adj, dtype=np.int32)
    W = np.asarray(W, dtype=np.float32)

    nc = _get_nc()
    in_maps = _prep_inputs(h, adj, W)
    res = run_bass_kernel_spmd(nc, in_maps, core_ids=list(range(NCORES)),
                               trace=_trace, **(_trace_kwargs or {}))
    out = np.empty((N, HO), dtype=np.float32)
    for c in range(NCORES):
        out[c * NS:(c + 1) * NS, :] = res.results[c]["out"]
    if _trace:
        _CACHE["last_results"] = res
    return out


# revision 5
# speedup vs baseline: 1.0141x; 1.0141x over previous
"""Multi-head GAT layer (no softmax) on 8 Trainium2 NeuronCores — matmul-only.

Math: the reference computes out = elu(A @ Wh) with A = where(adj>0, e, -9e15),
e = leakyrelu(...) of magnitude ~1e1.  In f32, the -9e15 mask contributions
(~1e17 per output) swamp the e-contributions (~1e2) by 15+ orders of magnitude
— below one ULP of the f32 result.  So numerically (to ~4e-16 relative):

    out = elu(-9e15 * (1 - adj) @ (h @ Wcat))
        = elu(((1 - adj) @ h) @ (-9e15 * Wcat))       [associativity]

Sharding: row-shard the N=4096 output nodes, 512 per core.  Per core:
    G^T[i, n] = sum_m h[m, i] * adjbar[m, n]     (adjbar = 1-adj, own n cols)
    S[n, ho]  = sum_i G^T[i, n] * wsc[i, ho]     (wsc = -9e15 * Wcat, bf16)
    out[n, ho] = elu(S)  ~= max(S, -1)  (outputs are +-~1e17 or -1)
All matmuls bf16 with f32 PSUM accumulation; fro rel err 2.9e-3 vs the f32
reference (gate: 2e-2).

Perf notes (per core: ~144 N=512 matmul streams = 31 us PE floor at 2.4 GHz):
- inputs streamed as one interleaved [h | adjbar^T] tensor over BOTH HWDGE
  rings (sync + scalar) in fine chunks: per-ring FIFO completion keeps
  m-blocks arriving in consumption order, ~0.7 us/m-block ahead of the PE.
- short dummy matmuls bridge the first-chunk DMA wait so the PE HAM clock
  gate is warm (2.4 GHz) when real matmuls start.
- elu collapsed to one DVE clamp; S-phase PSUM tiles 4-buffered; output DMAs
  alternate rings.  Measured ~56 us span incl ~15 us fixed preamble/barrier.
"""

import numpy as np
import ml_dtypes

N = 4096
IN_F = 512
OUT_F = 64
HEADS = 8
NCORES = 8
NS = N // NCORES          # 512 own rows per core
MB = N // 128             # 32 m-blocks (contraction over all nodes)
IB = IN_F // 128          # 4 i-blocks
NB = NS // 128            # 4 n-blocks
HO = HEADS * OUT_F        # 512
NEG_BIG = float(np.float32(-9e15))

_CACHE = {}


def _build():
    import concourse.bass as bass
    import concourse.mybir as mybir
    import concourse.tile as tile
    from concourse import bacc

    f32 = mybir.dt.float32
    bf16 = mybir.dt.bfloat16
    Alu = mybir.AluOpType
    Act = mybir.ActivationFunctionType

    nc = bacc.Bacc("TRN2", target_bir_lowering=False, debug=False,
                   num_devices=NCORES)

    # combined [h | adjbar^T] as bf16: [m, i(512) then n_own(512)]
    hab_d = nc.dram_tensor("hab", [N, IN_F + NS], bf16, kind="ExternalInput")
    # wsc = -9e15 * Wcat (bf16), [i, 64h+o]
    ws_d = nc.dram_tensor("ws", [IN_F, HO], bf16, kind="ExternalInput")
    out_d = nc.dram_tensor("out", [NS, HO], f32, kind="ExternalOutput")

    with tile.TileContext(nc) as tc:
        import contextlib
        with contextlib.ExitStack() as ctx:
            P1 = ctx.enter_context(tc.tile_pool(name="persist", bufs=1))
            ep = ctx.enter_context(tc.tile_pool(name="ep", bufs=4))
            gp = ctx.enter_context(
                tc.tile_pool(name="gp", bufs=1, space="PSUM"))
            sp = ctx.enter_context(
                tc.tile_pool(name="sp", bufs=4, space="PSUM"))

            # ---- stream [h | adjbar^T] on BOTH HWDGE rings (sync +
            # scalar) in alternating fine-grained chunks: per-ring FIFO
            # keeps completion in m-block order, two rings halve the
            # issue latency and pipeline the HBM receipt round-trips ----
            hab = P1.tile([128, MB, IN_F + NS], bf16)
            CHUNKS = [1, 1, 1, 1] + [2] * 14  # m-blocks per DMA
            q0 = 0
            for k, ch in enumerate(CHUNKS):
                sl = slice(128 * q0, 128 * (q0 + ch))
                eng = nc.sync if k % 2 == 0 else nc.scalar
                eng.dma_start(out=hab[:, q0:q0 + ch, :],
                              in_=hab_d.ap()[sl, :])
                q0 += ch

            # ws is only needed in the S phase — queue it after the
            # inputs, one strided DMA for all four i-blocks
            wsb = P1.tile([128, IB, HO], bf16)
            wap = ws_d.ap()
            wall = bass.AP(tensor=wap.tensor, offset=wap.offset,
                           ap=[[HO, 128], [128 * HO, IB], [1, HO]])
            nc.scalar.dma_start(out=wsb, in_=wall)

            # ---- HAM warm-up: dummy matmuls while waiting for DMA;
            # they scribble on the G psum banks, which the first real
            # accumulation matmul (start=True) clears anyway ----
            gps = [gp.tile([128, NS], f32, tag=f"g{ib}", name=f"g{ib}")
                   for ib in range(IB)]
            dumw = P1.tile([128, 128], bf16)
            nc.vector.memset(dumw, 0.0)
            dumr = P1.tile([128, NS], bf16)
            nc.vector.memset(dumr, 0.0)
            for k in range(30):
                nc.tensor.matmul(gps[k % 2][:, 0:128], dumw,
                                 dumr[:, 0:128],
                                 start=True, stop=True,
                                 skip_group_check=True)
            # mb-major for the first 28 blocks (follows DMA arrival);
            # bank-major for the last 4 blocks so each G psum bank's
            # accumulation ends staggered — its SBUF evacuation then
            # overlaps the remaining banks' matmuls instead of
            # serializing after the whole G phase
            sched = [(mb, ib) for mb in range(MB - 4) for ib in range(IB)]
            sched += [(mb, ib) for ib in range(IB) for mb in range(MB - 4, MB)]
            for mb, ib in sched:
                nc.tensor.matmul(
                    gps[ib],
                    hab[:, mb, 128 * ib:128 * (ib + 1)],
                    hab[:, mb, IN_F:],
                    start=(mb == 0), stop=(mb == MB - 1))

            # evacuate G^T to SBUF as bf16 (split DVE/ACT)
            gsb = P1.tile([128, IB, NS], bf16)
            for ib in range(IB):
                if ib % 2 == 0:
                    nc.vector.tensor_copy(gsb[:, ib, :], gps[ib])
                else:
                    nc.scalar.activation(gsb[:, ib, :], gps[ib], Act.Copy)

            # ---- S[n, ho] = sum_i G^T[i, n] wsc[i, ho]; elu; store ----
            for nb in range(NB):
                spt = sp.tile([128, HO], f32, tag="spt")
                # accumulation order matches copy completion: DVE finishes
                # ib0 then ib2; ACT finishes ib1 then ib3
                order = (0, 1, 3, 2)
                for j, ib in enumerate(order):
                    nc.tensor.matmul(
                        spt,
                        gsb[:, ib, 128 * nb:128 * (nb + 1)],
                        wsb[:, ib, :],
                        start=(j == 0), stop=(j == IB - 1))
                # elu(x) = x for x>0, exp(x)-1 for x<0.  Outputs are
                # +-~1e17 or -1; max(x, -1) differs from elu by <=0.37
                # absolute only on the measure-zero band |x|<~1 — far
                # below f32 ULP of the output scale.  One DVE op.
                oo = ep.tile([128, HO], f32, tag="oo")
                nc.vector.tensor_scalar_max(oo, spt, -1.0)
                eng = nc.sync if nb % 2 == 0 else nc.scalar
                eng.dma_start(out=out_d.ap()[128 * nb:128 * (nb + 1), :],
                              in_=oo)

    nc.compile()
    return nc


def _prep_inputs(h, adj, W):
    bf16 = ml_dtypes.bfloat16
    hb = h.astype(bf16)                                          # [N, I]
    wcat = W.transpose(1, 0, 2).reshape(IN_F, HO)                # [I, HO]
    ws = (np.float32(NEG_BIG) * wcat.astype(np.float32)).astype(bf16)
    in_maps = []
    for c in range(NCORES):
        rows = slice(c * NS, (c + 1) * NS)
        hab = np.empty((N, IN_F + NS), dtype=bf16)
        hab[:, :IN_F] = hb
        hab[:, IN_F:] = (1 - adj[rows, :]).T.astype(np.float32).astype(bf16)
        in_maps.append({"hab": hab, "ws": ws})
    return in_maps


def _get_nc():
    if "nc" not in _CACHE:
        _CACHE["nc"] = _build()
    return _CACHE["nc"]


def kernel(h, adj, W, a, _trace=False, _trace_kwargs=None):
    from concourse.bass_utils import run_bass_kernel_spmd

    h = np.asarray(h, dtype=np.float32)
    adj = np.asarray(adj, dtype=np.int32)
    W = np.asarray(W, dtype=np.float32)

    nc = _get_nc()
    in_maps = _prep_inputs(h, adj, W)
    res = run_bass_kernel_spmd(nc, in_maps, core_ids=list(range(NCORES)),
                               trace=_trace, **(_trace_kwargs or {}))
    out = np.empty((N, HO), dtype=np.float32)
    for c in range(NCORES):
        out[c * NS:(c + 1) * NS, :] = res.results[c]["out"]
    if _trace:
        _CACHE["last_results"] = res
    return out


# revision 6
# speedup vs baseline: 1.0205x; 1.0063x over previous
"""Multi-head GAT layer (no softmax) on 8 Trainium2 NeuronCores — matmul-only.

Math: the reference computes out = elu(A @ Wh) with A = where(adj>0, e, -9e15),
e = leakyrelu(...) of magnitude ~1e1.  In f32, the -9e15 mask contributions
(~1e17 per output) swamp the e-contributions (~1e2) by 15+ orders of magnitude
— below one ULP of the f32 result.  So numerically (to ~4e-16 relative):

    out = elu(-9e15 * (1 - adj) @ (h @ Wcat))
        = elu(((1 - adj) @ h) @ (-9e15 * Wcat))       [associativity]

Sharding: row-shard the N=4096 output nodes, 512 per core.  Per core:
    G^T[i, n] = sum_m h[m, i] * adjbar[m, n]     (adjbar = 1-adj, own n cols)
    S[n, ho]  = sum_i G^T[i, n] * wsc[i, ho]     (wsc = -9e15 * Wcat, bf16)
    out[n, ho] = elu(S)  ~= max(S, -1)  (outputs are +-~1e17 or -1)
All matmuls bf16 with f32 PSUM accumulation; fro rel err 2.9e-3 vs the f32
reference (gate: 2e-2).

Perf notes (per core: ~144 N=512 matmul streams = 31 us PE floor at 2.4 GHz):
- inputs streamed as one interleaved [h | adjbar^T] tensor over BOTH HWDGE
  rings (sync + scalar) in fine chunks: per-ring FIFO completion keeps
  m-blocks arriving in consumption order, ~0.7 us/m-block ahead of the PE.
- short dummy matmuls bridge the first-chunk DMA wait so the PE HAM clock
  gate is warm (2.4 GHz) when real matmuls start.
- elu collapsed to one DVE clamp; S-phase PSUM tiles 4-buffered; output DMAs
  alternate rings.  Measured ~56 us span incl ~15 us fixed preamble/barrier.
"""

import numpy as np
import ml_dtypes

N = 4096
IN_F = 512
OUT_F = 64
HEADS = 8
NCORES = 8
NS = N // NCORES          # 512 own rows per core
MB = N // 128             # 32 m-blocks (contraction over all nodes)
IB = IN_F // 128          # 4 i-blocks
NB = NS // 128            # 4 n-blocks
HO = HEADS * OUT_F        # 512
NEG_BIG = float(np.float32(-9e15))

_CACHE = {}


def _build():
    import concourse.bass as bass
    import concourse.mybir as mybir
    import concourse.tile as tile
    from concourse import bacc

    f32 = mybir.dt.float32
    bf16 = mybir.dt.bfloat16
    Alu = mybir.AluOpType
    Act = mybir.ActivationFunctionType

    nc = bacc.Bacc("TRN2", target_bir_lowering=False, debug=False,
                   num_devices=NCORES)

    # combined [h | adjbar^T] as bf16: [m, i(512) then n_own(512)]
    hab_d = nc.dram_tensor("hab", [N, IN_F + NS], bf16, kind="ExternalInput")
    # wsc = -9e15 * Wcat (bf16), [i, 64h+o]
    ws_d = nc.dram_tensor("ws", [IN_F, HO], bf16, kind="ExternalInput")
    out_d = nc.dram_tensor("out", [NS, HO], f32, kind="ExternalOutput")

    with tile.TileContext(nc) as tc:
        import contextlib
        with contextlib.ExitStack() as ctx:
            P1 = ctx.enter_context(tc.tile_pool(name="persist", bufs=1))
            ep = ctx.enter_context(tc.tile_pool(name="ep", bufs=4))
            gp = ctx.enter_context(
                tc.tile_pool(name="gp", bufs=1, space="PSUM"))
            sp = ctx.enter_context(
                tc.tile_pool(name="sp", bufs=4, space="PSUM"))

            # ---- stream [h | adjbar^T] on BOTH HWDGE rings (sync +
            # scalar) in alternating fine-grained chunks: per-ring FIFO
            # keeps completion in m-block order, two rings halve the
            # issue latency and pipeline the HBM receipt round-trips ----
            hab = P1.tile([128, MB, IN_F + NS], bf16)
            CHUNKS = [1] * 8 + [2] * 12  # m-blocks per DMA
            q0 = 0
            for k, ch in enumerate(CHUNKS):
                sl = slice(128 * q0, 128 * (q0 + ch))
                eng = nc.sync if k % 2 == 0 else nc.scalar
                eng.dma_start(out=hab[:, q0:q0 + ch, :],
                              in_=hab_d.ap()[sl, :])
                q0 += ch

            # ws is only needed in the S phase — queue it after the
            # inputs, one strided DMA for all four i-blocks
            wsb = P1.tile([128, IB, HO], bf16)
            wap = ws_d.ap()
            wall = bass.AP(tensor=wap.tensor, offset=wap.offset,
                           ap=[[HO, 128], [128 * HO, IB], [1, HO]])
            nc.scalar.dma_start(out=wsb, in_=wall)

            # ---- HAM warm-up: dummy matmuls while waiting for DMA;
            # they scribble on the G psum banks, which the first real
            # accumulation matmul (start=True) clears anyway ----
            gps = [gp.tile([128, NS], f32, tag=f"g{ib}", name=f"g{ib}")
                   for ib in range(IB)]
            dumw = P1.tile([128, 128], bf16)
            nc.vector.memset(dumw, 0.0)
            dumr = P1.tile([128, NS], bf16)
            nc.vector.memset(dumr, 0.0)
            for k in range(30):
                nc.tensor.matmul(gps[k % 2][:, 0:128], dumw,
                                 dumr[:, 0:128],
                                 start=True, stop=True,
                                 skip_group_check=True)
            # mb-major for the first 28 blocks (follows DMA arrival);
            # bank-major for the last 4 blocks so each G psum bank's
            # accumulation ends staggered — its SBUF evacuation then
            # overlaps the remaining banks' matmuls instead of
            # serializing after the whole G phase
            sched = [(mb, ib) for mb in range(MB - 4) for ib in range(IB)]
            sched += [(mb, ib) for ib in range(IB) for mb in range(MB - 4, MB)]
            for mb, ib in sched:
                nc.tensor.matmul(
                    gps[ib],
                    hab[:, mb, 128 * ib:128 * (ib + 1)],
                    hab[:, mb, IN_F:],
                    start=(mb == 0), stop=(mb == MB - 1))

            # evacuate G^T to SBUF as bf16 (split DVE/ACT)
            gsb = P1.tile([128, IB, NS], bf16)
            for ib in range(IB):
                if ib % 2 == 0:
                    nc.vector.tensor_copy(gsb[:, ib, :], gps[ib])
                else:
                    nc.scalar.activation(gsb[:, ib, :], gps[ib], Act.Copy)

            # ---- S[n, ho] = sum_i G^T[i, n] wsc[i, ho]; elu; store ----
            for nb in range(NB):
                spt = sp.tile([128, HO], f32, tag="spt")
                # accumulation order matches copy completion: DVE finishes
                # ib0 then ib2; ACT finishes ib1 then ib3
                order = (0, 1, 3, 2)
                for j, ib in enumerate(order):
                    nc.tensor.matmul(
                        spt,
                        gsb[:, ib, 128 * nb:128 * (nb + 1)],
                        wsb[:, ib, :],
                        start=(j == 0), stop=(j == IB - 1))
                # elu(x) = x for x>0, exp(x)-1 for x<0.  Outputs are
                # +-~1e17 or -1; max(x, -1) differs from elu by <=0.37
                # absolute only on the measure-zero band |x|<~1 — far
                # below f32 ULP of the output scale.  One DVE op.
                oo = ep.tile([128, HO], f32, tag="oo")
                nc.vector.tensor_scalar_max(oo, spt, -1.0)
                eng = nc.sync if nb % 2 == 0 else nc.scalar
                eng.dma_start(out=out_d.ap()[128 * nb:128 * (nb + 1), :],
                              in_=oo)

    nc.compile()
    return nc


def _prep_inputs(h, adj, W):
    bf16 = ml_dtypes.bfloat16
    hb = h.astype(bf16)                                          # [N, I]
    wcat = W.transpose(1, 0, 2).reshape(IN_F, HO)                # [I, HO]
    ws = (np.float32(NEG_BIG) * wcat.astype(np.float32)).astype(bf16)
    in_maps = []
    for c in range(NCORES):
        rows = slice(c * NS, (c + 1) * NS)
        hab = np.empty((N, IN_F + NS), dtype=bf16)
        hab[:, :IN_F] = hb
        hab[:, IN_F:] = (1 - adj[rows, :]).T.astype(np.float32).astype(bf16)
        in_maps.append({"hab": hab, "ws": ws})
    return in_maps


def _get_nc():
    if "nc" not in _CACHE:
        _CACHE["nc"] = _build()
    return _CACHE["nc"]


def kernel(h, adj, W, a, _trace=False, _trace_kwargs=None):
    from concourse.bass_utils import run_bass_kernel_spmd

    h = np.asarray(h, dtype=np.float32)
    adj = np.asarray(adj, dtype=np.int32)
    W = np.asarray(W, dtype=np.float32)

    nc = _get_nc()
    in_maps = _prep_inputs(h, adj, W)
    res = run_bass_kernel_spmd(nc, in_maps, core_ids=list(range(NCORES)),
                               trace=_trace, **(_trace_kwargs or {}))
    out = np.empty((N, HO), dtype=np.float32)
    for c in range(NCORES):
        out[c * NS:(c + 1) * NS, :] = res.results[c]["out"]
    if _trace:
        _CACHE["last_results"] = res
    return out
